# revision 1
# baseline (speedup 1.0000x reference)
"""Self-contained DiT forward kernel for 8 TRN2 NeuronCores.

Sharding: data-parallel over the batch (32 images -> 4 per core); all weights
replicated per core and streamed layer-by-layer from HBM.

Device kernel design (feature-major activations X^T [feature, token]):
- Dense GEMMs form 1: out^T[m,tok] = matmul(lhsT=W[k,m], rhs=X^T[k,tok])
- V token-major via form 2: V[tok,feat] = matmul(lhsT=X^T[k,tok_tile], rhs=W[k,feat])
- Attention: S^T[Tk,Tq] = matmul(lhsT=K^T[dh,Tk_tile], rhs=Q^T[dh,Tq]); exp on ACT
  (no max-subtraction: logits are bounded ~|2| for this model family);
  softmax denominator via ones-matmul; normalization via K=1 PE broadcast of 1/denom.
- LayerNorm over features via ones-matmul partition sums + PE broadcast of
  mean/rstd; adaLN modulation via ACT scale/bias [P,1] APs (per image, feature).
- Dtypes: float32r (full-rate fp32 PE mode) everywhere except the two MLP GEMMs
  which run in bf16 with fp32 PSUM accumulation.
Host side: patchify/unpatchify (pure data movement), timestep embedding + class
embedding lookup + silu on the [32,1024] conditioning vector, bf16 weight casts.
"""
import sys
sys.path.insert(0, "/opt/trn_rl_repo")
from contextlib import ExitStack

import numpy as np
import ml_dtypes

import concourse.bass as bass
import concourse.mybir as mybir
import concourse.tile as tile
from concourse import bacc

F32 = mybir.dt.float32
F32R = mybir.dt.float32r
BF16 = mybir.dt.bfloat16
AF = mybir.ActivationFunctionType
ALU = mybir.AluOpType

# model dims (hardcoded for nn_DiT_8529805050403)
B_FULL, D_CH, H_IMG, W_IMG = 32, 64, 32, 32
PATCH = 2
LFEAT = 256
TPI = 256
DH = 64
HPK = 2
N_CORES = 8
HS, NH, NL = 1024, 16, 12
NC_CLS = 1000


class Cfg:
    def __init__(self, NIMG=4, HS=1024, NH=16, NL=12):
        self.NIMG, self.HS, self.NH, self.NL = NIMG, HS, NH, NL
        self.DFF = 4 * HS
        self.T = NIMG * TPI
        self.KT = HS // 128
        self.GKT = self.DFF // 128
        self.CW = min(512, self.T)
        self.NCH = self.T // self.CW
        self.IPC = self.CW // TPI
        assert HS % 128 == 0 and self.T % self.CW == 0 and NH == HS // DH


def build_dit(c: Cfg):
    nc = bacc.Bacc("TRN2", target_bir_lowering=False, debug=False)

    dt_ = nc.dram_tensor
    tokT_d = dt_("tokT", [LFEAT, c.T], F32R, kind="ExternalInput")
    posT_d = dt_("posT", [c.HS, TPI], F32R, kind="ExternalInput")
    cactT_d = dt_("cactT", [c.HS, c.NIMG], F32R, kind="ExternalInput")
    ident_d = dt_("ident", [128, 128], F32R, kind="ExternalInput")
    ones_d = dt_("ones", [128, 512], F32R, kind="ExternalInput")
    onesb_d = dt_("ones_bf", [1, 512], BF16, kind="ExternalInput")
    projw_d = dt_("proj_w", [LFEAT, c.HS], F32R, kind="ExternalInput")
    projb_d = dt_("proj_b", [c.HS], F32, kind="ExternalInput")
    modw_d = dt_("mod_w", [c.NL, c.HS, 6 * c.HS], F32R, kind="ExternalInput")
    modb_d = dt_("mod_b", [c.NL, 1, 6 * c.HS], F32R, kind="ExternalInput")
    wq_d = dt_("wq", [c.NL, c.HS, c.HS], F32R, kind="ExternalInput")
    wk_d = dt_("wk", [c.NL, c.HS, c.HS], F32R, kind="ExternalInput")
    wv_d = dt_("wv", [c.NL, c.HS, c.HS], F32R, kind="ExternalInput")
    wo_d = dt_("wo", [c.NL, c.HS, c.HS], F32R, kind="ExternalInput")
    bq_d = dt_("bq", [c.NL, c.HS], F32, kind="ExternalInput")
    bk_d = dt_("bk", [c.NL, c.HS], F32, kind="ExternalInput")
    bv_d = dt_("bv", [c.NL, 1, c.HS], F32R, kind="ExternalInput")
    bo_d = dt_("bo", [c.NL, 1, c.HS], F32R, kind="ExternalInput")
    f1w_d = dt_("f1w", [c.NL, c.HS, c.DFF], BF16, kind="ExternalInput")
    f1b_d = dt_("f1b", [c.NL, c.DFF], F32, kind="ExternalInput")
    f2w_d = dt_("f2w", [c.NL, c.DFF, c.HS], BF16, kind="ExternalInput")
    f2b_d = dt_("f2b", [c.NL, 1, c.HS], BF16, kind="ExternalInput")
    fmodw_d = dt_("fmod_w", [c.HS, 2 * c.HS], F32R, kind="ExternalInput")
    fmodb_d = dt_("fmod_b", [1, 2 * c.HS], F32R, kind="ExternalInput")
    foutw_d = dt_("fout_w", [c.HS, LFEAT], F32R, kind="ExternalInput")
    foutb_d = dt_("fout_b", [LFEAT], F32, kind="ExternalInput")
    outT_d = dt_("outT", [LFEAT, c.T], F32, kind="ExternalOutput")

    with tile.TileContext(nc) as tc, ExitStack() as ctx:
        def pool(name, bufs, **kw):
            return ctx.enter_context(tc.tile_pool(name=name, bufs=bufs, **kw))
        const = pool("const", 1)
        resid = pool("resid", c.KT)
        hxp = pool("hxp", 1)
        qkp = pool("qkp", 2)
        vop = pool("vop", 2)
        gp = pool("gp", 1)
        modp = pool("modp", 1)
        w5p = pool("w5p", 4)
        wbp = pool("wbp", 4)
        biasp = pool("biasp", 1)
        tmpp = pool("tmpp", 2)
        rowp = pool("rowp", 1)
        pexpp = pool("pexpp", 2)
        outpp = pool("outpp", 2)
        mmp = pool("mmp", 4, space="PSUM")
        apsp = pool("apsp", 2, space="PSUM")
        spsp = pool("spsp", 2, space="PSUM")

        ident = const.tile([128, 128], F32R, tag="ident")
        nc.sync.dma_start(out=ident, in_=ident_d.ap())
        ones = const.tile([128, 512], F32R, tag="ones")
        nc.sync.dma_start(out=ones, in_=ones_d.ap())
        ones_bf = const.tile([1, 512], BF16, tag="ones_bf")
        nc.sync.dma_start(out=ones_bf, in_=onesb_d.ap())
        cact_sb = const.tile([128, c.KT, c.NIMG], F32R, tag="cact")
        nc.sync.dma_start(out=cact_sb,
                          in_=cactT_d.ap().rearrange("(kt p) i -> p kt i", p=128))
        pb_sb = const.tile([128, c.KT], F32, tag="pb")
        nc.sync.dma_start(out=pb_sb,
                          in_=projb_d.ap().rearrange("(kt p) -> p kt", p=128))
        fob_sb = const.tile([128, LFEAT // 128], F32, tag="fob")
        nc.sync.dma_start(out=fob_sb,
                          in_=foutb_d.ap().rearrange("(kt p) -> p kt", p=128))
        tok_sb = qkp.tile([128, LFEAT // 128, c.T], F32R, tag="qkc")
        nc.sync.dma_start(out=tok_sb,
                          in_=tokT_d.ap().rearrange("(kt p) t -> p kt t", p=128))
        pos_sb = vop.tile([128, c.KT, TPI], F32R, tag="voc")
        nc.sync.dma_start(out=pos_sb,
                          in_=posT_d.ap().rearrange("(kt p) t -> p kt t", p=128))
        eps_sb = const.tile([1, 1], F32, tag="eps")
        nc.vector.memset(eps_sb, 1e-5)

        X = [resid.tile([128, c.T], F32R, tag="X", name=f"X{ft}")
             for ft in range(c.KT)]

        def gemm_form1(w_ap, rhs_fn, nk, m_tiles, nw, evac_fn, wpool, wdt,
                       bias_row=None, ones_row=None, mgrp=4, wtag="w"):
            for g0 in range(0, m_tiles, mgrp):
                gsz = min(mgrp, m_tiles - g0)
                psums = [mmp.tile([128, 512], F32, tag="mm", name=f"mm{_i}")
                         for _i in range(gsz)]
                for k in range(nk):
                    wc = wpool.tile([128, mgrp * 128], wdt, tag=wtag)
                    nc.sync.dma_start(
                        out=wc[:, :gsz * 128],
                        in_=w_ap[k * 128:(k + 1) * 128,
                                 g0 * 128:(g0 + gsz) * 128])
                    for mi in range(gsz):
                        nc.tensor.matmul(
                            psums[mi][:, :nw], wc[:, mi * 128:(mi + 1) * 128],
                            rhs_fn(k), start=(k == 0),
                            stop=(k == nk - 1 and bias_row is None))
                if bias_row is not None:
                    for mi in range(gsz):
                        nc.tensor.matmul(
                            psums[mi][:, :nw],
                            bias_row[0:1, (g0 + mi) * 128:(g0 + mi + 1) * 128],
                            ones_row[0:1, 0:nw], start=False, stop=True)
                for mi in range(gsz):
                    evac_fn(g0 + mi, psums[mi][:, :nw])

        def layernorm_mod(cols0, cw, ipc, modsl, out_fn):
            ps_s = spsp.tile([1, 512], F32, tag="sps")
            ps_q = spsp.tile([1, 512], F32, tag="sps")
            for ft in range(c.KT):
                xs = X[ft][:, cols0:cols0 + cw]
                sq = tmpp.tile([128, 512], F32R, tag="xsq")
                nc.scalar.activation(sq[:, :cw], xs, AF.Square)
                nc.tensor.matmul(ps_s[:, :cw], ones[:, 0:1], xs,
                                 start=(ft == 0), stop=(ft == c.KT - 1))
                nc.tensor.matmul(ps_q[:, :cw], ones[:, 0:1], sq[:, :cw],
                                 start=(ft == 0), stop=(ft == c.KT - 1))
            mean = rowp.tile([1, 512], F32R, tag="mean")
            nc.scalar.activation(mean[:, :cw], ps_s[:, :cw], AF.Copy,
                                 scale=1.0 / c.HS)
            mu_bc = apsp.tile([128, 512], F32, tag="aps")
            nc.tensor.matmul(mu_bc[:, :cw], ones[0:1, 0:128], mean[:, :cw],
                             start=True, stop=True)
            msq = rowp.tile([1, 512], F32, tag="msq")
            nc.scalar.activation(msq[:, :cw], mean[:, :cw], AF.Square)
            var = rowp.tile([1, 512], F32, tag="var")
            nc.vector.scalar_tensor_tensor(
                var[:, :cw], ps_q[:, :cw], 1.0 / c.HS, msq[:, :cw],
                op0=ALU.mult, op1=ALU.subtract)
            std = rowp.tile([1, 512], F32, tag="std")
            nc.scalar.activation(std[:, :cw], var[:, :cw], AF.Sqrt,
                                 bias=eps_sb[0:1, 0:1])
            rstd = rowp.tile([1, 512], F32R, tag="rstd")
            with nc.allow_low_precision(reason="f32r storage is fp32"):
                nc.vector.reciprocal(rstd[:, :cw], std[:, :cw])
            rs_bc = apsp.tile([128, 512], F32, tag="aps")
            nc.tensor.matmul(rs_bc[:, :cw], ones[0:1, 0:128], rstd[:, :cw],
                             start=True, stop=True)
            for ft in range(c.KT):
                xs = X[ft][:, cols0:cols0 + cw]
                t1 = tmpp.tile([128, 512], F32, tag="t1")
                nc.vector.tensor_sub(t1[:, :cw], xs, mu_bc[:, :cw])
                for i in range(ipc):
                    sc_ap, sh_ap = modsl(ft, i)
                    t2 = tmpp.tile([128, TPI], F32, tag="t2")
                    nc.vector.scalar_tensor_tensor(
                        t2, t1[:, i * TPI:(i + 1) * TPI], sc_ap,
                        rs_bc[:, i * TPI:(i + 1) * TPI],
                        op0=ALU.mult, op1=ALU.mult)
                    nc.scalar.activation(out_fn(ft, i), t2, AF.Identity,
                                         bias=sh_ap)

        def mod_gemm(w_ap, b_ap, nout, dest):
            for ch in range(nout // 512):
                b_sb = rowp.tile([1, 512], F32R, tag="modb")
                nc.sync.dma_start(out=b_sb,
                                  in_=b_ap[0:1, ch * 512:(ch + 1) * 512])
                psm = spsp.tile([c.NIMG, 512], F32, tag="sps")
                for k in range(c.KT):
                    wc = w5p.tile([128, 512], F32R, tag="w")
                    nc.sync.dma_start(
                        out=wc, in_=w_ap[k * 128:(k + 1) * 128,
                                         ch * 512:(ch + 1) * 512])
                    nc.tensor.matmul(psm, cact_sb[:, k, :], wc,
                                     start=(k == 0), stop=False)
                nc.tensor.matmul(psm, ones[0:1, 0:c.NIMG], b_sb,
                                 start=False, stop=True)
                MTP = max(c.NIMG, 4)
                mtok = rowp.tile([MTP, 512], F32, tag="mtok")
                if MTP > c.NIMG:
                    nc.vector.memset(mtok, 0.0)
                nc.scalar.activation(mtok[0:c.NIMG, :], psm, AF.Copy)
                for j in range(4):
                    pst = apsp.tile([128, MTP], F32, tag="aps")
                    nc.tensor.transpose(pst, mtok[:, j * 128:(j + 1) * 128],
                                        ident[0:MTP, 0:MTP].bitcast(F32))
                    nc.scalar.activation(dest[:, ch * 4 + j, :],
                                         pst[:, 0:c.NIMG], AF.Copy)

        # ---------------- patchify projection ----------------
        for chn in range(c.NCH):
            cc = chn * c.CW

            def ev_p(mt, ps, _cc=cc):
                nc.scalar.activation(X[mt][:, _cc:_cc + c.CW], ps, AF.Identity,
                                     bias=pb_sb[:, mt:mt + 1])
                for i in range(c.IPC):
                    cs = _cc + i * TPI
                    nc.vector.tensor_add(X[mt][:, cs:cs + TPI],
                                         X[mt][:, cs:cs + TPI], pos_sb[:, mt, :])
            gemm_form1(projw_d.ap(),
                       lambda k, _cc=cc: tok_sb[:, k, _cc:_cc + c.CW],
                       LFEAT // 128, c.KT, c.CW, ev_p, w5p, F32R)

        # ---------------- transformer layers ----------------
        for l in range(c.NL):
            bqT = biasp.tile([128, c.KT], F32, tag="bqT")
            nc.sync.dma_start(out=bqT,
                              in_=bq_d.ap()[l].rearrange("(kt p) -> p kt", p=128))
            bkT = biasp.tile([128, c.KT], F32, tag="bkT")
            nc.sync.dma_start(out=bkT,
                              in_=bk_d.ap()[l].rearrange("(kt p) -> p kt", p=128))
            bv_sb = biasp.tile([1, c.HS], F32R, tag="bv")
            nc.sync.dma_start(out=bv_sb, in_=bv_d.ap()[l])
            bo_sb = biasp.tile([1, c.HS], F32R, tag="bo")
            nc.sync.dma_start(out=bo_sb, in_=bo_d.ap()[l])
            f1bT = biasp.tile([128, c.GKT], F32, tag="f1bT")
            nc.sync.dma_start(out=f1bT,
                              in_=f1b_d.ap()[l].rearrange("(kt p) -> p kt", p=128))
            f2b_sb = biasp.tile([1, c.HS], BF16, tag="f2b")
            nc.sync.dma_start(out=f2b_sb, in_=f2b_d.ap()[l])

            modT = modp.tile([128, 6 * c.KT, c.NIMG], F32, tag="modT")
            mod_gemm(modw_d.ap()[l], modb_d.ap()[l], 6 * c.HS, modT)
            nc.vector.tensor_scalar_add(modT[:, c.KT:2 * c.KT, :],
                                        modT[:, c.KT:2 * c.KT, :], 1.0)
            nc.vector.tensor_scalar_add(modT[:, 4 * c.KT:5 * c.KT, :],
                                        modT[:, 4 * c.KT:5 * c.KT, :], 1.0)

            for chn in range(c.NCH):
                cc = chn * c.CW

                hx1 = hxp.tile([128, c.KT, c.CW], F32R, tag="hx")
                def msl_a(ft, i, _chn=chn, _m=modT):
                    gi = _chn * c.IPC + i
                    return (_m[:, c.KT + ft, gi:gi + 1], _m[:, ft, gi:gi + 1])
                layernorm_mod(cc, c.CW, c.IPC, msl_a,
                              lambda ft, i: hx1[:, ft, i * TPI:(i + 1) * TPI])

                Qc = qkp.tile([128, c.KT, c.CW], F32R, tag="qkc")
                Kc = qkp.tile([128, c.KT, c.CW], F32R, tag="qkc")
                for (w_ap, dst, bT) in ((wq_d.ap()[l], Qc, bqT),
                                        (wk_d.ap()[l], Kc, bkT)):
                    def ev_qk(mt, ps, _dst=dst, _bT=bT):
                        nc.scalar.activation(_dst[:, mt, :], ps, AF.Identity,
                                             bias=_bT[:, mt:mt + 1])
                    gemm_form1(w_ap, lambda k, _h=hx1: _h[:, k, :], c.KT, c.KT,
                               c.CW, ev_qk, w5p, F32R)
                Vc = vop.tile([128, c.CW // 128, c.HS], F32R, tag="voc")
                FOW = min(512, c.HS)
                for tt in range(c.CW // 128):
                    for fo in range(c.HS // FOW):
                        psv = mmp.tile([128, 512], F32, tag="mm")
                        for k in range(c.KT):
                            wc = w5p.tile([128, 512], F32R, tag="w")
                            nc.sync.dma_start(
                                out=wc[:, :FOW],
                                in_=wv_d.ap()[l][k * 128:(k + 1) * 128,
                                                 fo * FOW:(fo + 1) * FOW])
                            nc.tensor.matmul(psv[:, :FOW],
                                             hx1[:, k, tt * 128:(tt + 1) * 128],
                                             wc[:, :FOW], start=(k == 0),
                                             stop=False)
                        nc.tensor.matmul(psv[:, :FOW], ones[0:1, 0:128],
                                         bv_sb[0:1, fo * FOW:(fo + 1) * FOW],
                                         start=False, stop=True)
                        nc.scalar.activation(Vc[:, tt, fo * FOW:(fo + 1) * FOW],
                                             psv[:, :FOW], AF.Copy)

                Oc = vop.tile([128, c.KT, c.CW], F32R, tag="voc")
                for i in range(c.IPC):
                    for h in range(c.NH):
                        kt, r0 = h // HPK, DH * (h % HPK)
                        i0 = i * TPI
                        s_ps = mmp.tile([128, 2, TPI], F32, tag="mm")
                        for tk in range(2):
                            nc.tensor.matmul(
                                s_ps[:, tk, :],
                                Kc[r0:r0 + DH, kt,
                                   i0 + tk * 128:i0 + (tk + 1) * 128],
                                Qc[r0:r0 + DH, kt, i0:i0 + TPI],
                                start=True, stop=True)
                        p_sb = pexpp.tile([128, 2, TPI], F32R, tag="pexp")
                        nc.scalar.activation(p_sb, s_ps, AF.Exp, scale=0.125)
                        d_ps = spsp.tile([1, TPI], F32, tag="sps")
                        for tk in range(2):
                            nc.tensor.matmul(d_ps, ones[:, 0:1], p_sb[:, tk, :],
                                             start=(tk == 0), stop=(tk == 1))
                        rec = rowp.tile([1, TPI], F32R, tag="rec")
                        with nc.allow_low_precision(reason="f32r storage is fp32"):
                            nc.vector.reciprocal(rec, d_ps)
                        u_ps = mmp.tile([128, TPI], F32, tag="mm")
                        for tk in range(2):
                            nc.tensor.matmul(
                                u_ps[0:DH, :],
                                Vc[:, i * 2 + tk, h * DH:(h + 1) * DH],
                                p_sb[:, tk, :], start=(tk == 0), stop=(tk == 1))
                        bc_ps = apsp.tile([128, TPI], F32, tag="aps")
                        nc.tensor.matmul(bc_ps[0:DH, :], ones[0:1, 0:DH], rec,
                                         start=True, stop=True)
                        bc_sb = pexpp.tile([128, TPI], F32, tag="bcsb")
                        nc.scalar.activation(bc_sb[0:DH, :], bc_ps[0:DH, :],
                                             AF.Copy)
                        nc.vector.tensor_mul(
                            Oc[r0:r0 + DH, kt, i0:i0 + TPI],
                            u_ps[0:DH, :], bc_sb[0:DH, :])

                def ev_o(mt, ps, _chn=chn, _cc=cc, _m=modT):
                    for i in range(c.IPC):
                        gi = _chn * c.IPC + i
                        xa = X[mt][:, _cc + i * TPI:_cc + (i + 1) * TPI]
                        nc.vector.scalar_tensor_tensor(
                            xa, ps[:, i * TPI:(i + 1) * TPI],
                            _m[:, 2 * c.KT + mt, gi:gi + 1], xa,
                            op0=ALU.mult, op1=ALU.add)
                gemm_form1(wo_d.ap()[l], lambda k, _o=Oc: _o[:, k, :], c.KT,
                           c.KT, c.CW, ev_o, w5p, F32R,
                           bias_row=bo_sb, ones_row=ones)

                hx2 = hxp.tile([128, c.KT, c.CW], BF16, tag="hx")
                def msl_m(ft, i, _chn=chn, _m=modT):
                    gi = _chn * c.IPC + i
                    return (_m[:, 4 * c.KT + ft, gi:gi + 1],
                            _m[:, 3 * c.KT + ft, gi:gi + 1])
                layernorm_mod(cc, c.CW, c.IPC, msl_m,
                              lambda ft, i: hx2[:, ft, i * TPI:(i + 1) * TPI])

                g = gp.tile([128, c.GKT, c.CW], BF16, tag="g")
                def ev_g(mt, ps, _g=g):
                    nc.scalar.activation(_g[:, mt, :], ps, AF.Gelu,
                                         bias=f1bT[:, mt:mt + 1])
                gemm_form1(f1w_d.ap()[l], lambda k, _h=hx2: _h[:, k, :], c.KT,
                           c.GKT, c.CW, ev_g, wbp, BF16, wtag="wb")
                def ev_m(mt, ps, _chn=chn, _cc=cc, _m=modT):
                    for i in range(c.IPC):
                        gi = _chn * c.IPC + i
                        xa = X[mt][:, _cc + i * TPI:_cc + (i + 1) * TPI]
                        nc.vector.scalar_tensor_tensor(
                            xa, ps[:, i * TPI:(i + 1) * TPI],
                            _m[:, 5 * c.KT + mt, gi:gi + 1], xa,
                            op0=ALU.mult, op1=ALU.add)
                gemm_form1(f2w_d.ap()[l], lambda k, _g=g: _g[:, k, :], c.GKT,
                           c.KT, c.CW, ev_m, wbp, BF16,
                           bias_row=f2b_sb, ones_row=ones_bf, wtag="wb")

        # ---------------- final layer ----------------
        fmodT = modp.tile([128, 2 * c.KT, c.NIMG], F32, tag="modT")
        mod_gemm(fmodw_d.ap(), fmodb_d.ap(), 2 * c.HS, fmodT)
        nc.vector.tensor_scalar_add(fmodT[:, c.KT:2 * c.KT, :],
                                    fmodT[:, c.KT:2 * c.KT, :], 1.0)
        for chn in range(c.NCH):
            cc = chn * c.CW
            hxf = hxp.tile([128, c.KT, c.CW], F32R, tag="hx")
            def msl_f(ft, i, _chn=chn, _m=fmodT):
                gi = _chn * c.IPC + i
                return (_m[:, c.KT + ft, gi:gi + 1], _m[:, ft, gi:gi + 1])
            layernorm_mod(cc, c.CW, c.IPC, msl_f,
                          lambda ft, i: hxf[:, ft, i * TPI:(i + 1) * TPI])
            oc = outpp.tile([128, c.CW], F32, tag="oc")
            oc2 = outpp.tile([128, c.CW], F32, tag="oc")
            ocs = [oc, oc2]
            def ev_f(mt, ps, _ocs=ocs):
                nc.scalar.activation(_ocs[mt], ps, AF.Identity,
                                     bias=fob_sb[:, mt:mt + 1])
            gemm_form1(foutw_d.ap(), lambda k, _h=hxf: _h[:, k, :], c.KT,
                       LFEAT // 128, c.CW, ev_f, w5p, F32R, mgrp=2)
            for mt in range(LFEAT // 128):
                nc.sync.dma_start(
                    out=outT_d.ap()[mt * 128:(mt + 1) * 128, cc:cc + c.CW],
                    in_=ocs[mt])

    nc.compile()
    return nc


# ---------------- host-side pre/post-processing ----------------

def _timestep_cond(t, y, emb_table):
    """c_act = silu(timestep_embedding(t) + emb_table[y])  [B, HS] f32."""
    half = HS // 2
    freqs = np.exp(-np.log(np.float32(10000.0)) *
                   np.arange(half, dtype=np.float32) / np.float32(half))
    ang = t.astype(np.float32)[:, None] * freqs[None]
    cemb = np.concatenate([np.cos(ang), np.sin(ang)], axis=-1)
    cc = cemb + np.asarray(emb_table, np.float32)[np.asarray(y).astype(np.int64)]
    return (cc / (1.0 + np.exp(-cc))).astype(np.float32)


def _patchify(x):
    """x [B, D, H, W] -> tokens [B, TPI, LFEAT] (f32)."""
    Bc = x.shape[0]
    Hp, Wp = H_IMG // PATCH, W_IMG // PATCH
    return np.ascontiguousarray(
        x.reshape(Bc, D_CH, Hp, PATCH, Wp, PATCH)
         .transpose(0, 2, 4, 3, 5, 1).reshape(Bc, Hp * Wp, LFEAT))


def _unpatchify(tokens):
    """tokens [B, TPI, LFEAT] -> [B, D, H, W]."""
    Bc = tokens.shape[0]
    Hp, Wp = H_IMG // PATCH, W_IMG // PATCH
    return np.ascontiguousarray(
        tokens.reshape(Bc, Hp, Wp, PATCH, PATCH, D_CH)
              .transpose(0, 5, 1, 3, 2, 4).reshape(Bc, D_CH, H_IMG, W_IMG))


_CACHE = {}


def _get_nc():
    if "nc" not in _CACHE:
        _CACHE["nc"] = build_dit(Cfg(NIMG=B_FULL // N_CORES, HS=HS, NH=NH, NL=NL))
    return _CACHE["nc"]


def kernel(x, y, t, proj_w, proj_b, pos_embed, emb_table,
           blk_mod_w, blk_mod_b, blk_wq, blk_bq, blk_wk, blk_bk, blk_wv, blk_bv,
           blk_wo, blk_bo, blk_fc1_w, blk_fc1_b, blk_fc2_w, blk_fc2_b,
           fin_mod_w, fin_mod_b, fin_out_w, fin_out_b):
    from concourse import bass_utils

    bf = ml_dtypes.bfloat16
    f = np.float32
    x = np.asarray(x, f)
    nimg = B_FULL // N_CORES

    tok = _patchify(x)                       # [B, TPI, LFEAT]
    c_act = _timestep_cond(np.asarray(t), np.asarray(y), emb_table)  # [B, HS]

    shared = {
        'posT': np.ascontiguousarray(np.asarray(pos_embed, f).T),
        'ident': np.eye(128, dtype=f),
        'ones': np.ones((128, 512), f),
        'ones_bf': np.ones((1, 512), bf),
        'proj_w': np.asarray(proj_w, f), 'proj_b': np.asarray(proj_b, f),
        'mod_w': np.asarray(blk_mod_w, f),
        'mod_b': np.asarray(blk_mod_b, f)[:, None, :],
        'wq': np.asarray(blk_wq, f), 'wk': np.asarray(blk_wk, f),
        'wv': np.asarray(blk_wv, f), 'wo': np.asarray(blk_wo, f),
        'bq': np.asarray(blk_bq, f), 'bk': np.asarray(blk_bk, f),
        'bv': np.asarray(blk_bv, f)[:, None, :],
        'bo': np.asarray(blk_bo, f)[:, None, :],
        'f1w': np.asarray(blk_fc1_w, f).astype(bf),
        'f1b': np.asarray(blk_fc1_b, f),
        'f2w': np.asarray(blk_fc2_w, f).astype(bf),
        'f2b': np.asarray(blk_fc2_b, f).astype(bf)[:, None, :],
        'fmod_w': np.asarray(fin_mod_w, f),
        'fmod_b': np.asarray(fin_mod_b, f)[None],
        'fout_w': np.asarray(fin_out_w, f), 'fout_b': np.asarray(fin_out_b, f),
    }
    in_maps = []
    for cid in range(N_CORES):
        sl = slice(cid * nimg, (cid + 1) * nimg)
        im = dict(shared)
        im['tokT'] = np.ascontiguousarray(
            tok[sl].reshape(nimg * TPI, LFEAT).T)
        im['cactT'] = np.ascontiguousarray(c_act[sl].T)
        in_maps.append(im)

    nc = _get_nc()
    res = bass_utils.run_bass_kernel_spmd(nc, in_maps,
                                          core_ids=list(range(N_CORES)))
    toks_out = np.concatenate(
        [res.results[cid]['outT'].T.reshape(nimg, TPI, LFEAT)
         for cid in range(N_CORES)], axis=0)
    return _unpatchify(toks_out).astype(f)

